# revision 21
# baseline (speedup 1.0000x reference)
"""Cross-attention Trainium2 Bass kernel (v3, 16-bit datapath, fused pipeline).

Math (per batch element b, one per NeuronCore):
    q = x Wq + bq            [Sq, 8]
    k = ctx Wk + bk          [Sk, 8]
    v = ctx Wv + bv          [Sk, 8]
    scores = q k^T           [Sq, Sk]
    w = softmax(scores)      (no max subtraction; scores are bounded ~|31|)
    out = w v                [Sq, 8]
    y = out Wo + bo          [Sq, 1024]

Dtypes: x/ctx/q/k fp16 (halves input DMA, keeps ~f32r-grade scores),
exp/v/out bf16 (exp needs e^31 range), y fp16 out (host upcasts).
Measured end-to-end rel err ~7e-3 vs f64 reference (gate 2e-2).

Layout (all contractions need the contracted dim on SBUF partitions, so
x/ctx are fed pre-transposed as xT/ctxT [1024, 2048]):
    kv_ps[40, t] = Wkv^T ctxT   (one fused pass; v rows 0-7, k rows 32-39)
    qT[40, s]    = Wq^T xT      (q on partitions 32-39, same PE row group
                                 as k so the scores matmul bases match)
    v_ext[t, 33] = PE-transpose of [vT; ones; 0...; ones] (ones rows 8, 32)
    E^T[t, s]    = exp(kT^T qT)          (scoresT via PE, exp via ScalarE)
    oe[33, s]    = v_ext^T E^T  accumulated over t-chunks:
        rows 0-7 = sum_t E v    rows 8, 32 = sum_t E  (softmax denominator)
    y[s, d]      = [out; den]^T [Wo; bo]  scaled by 1/den per-partition.

Scheduling: ScalarE (ACT) does nothing but the 32 exp instructions
(1 elem/cycle/lane @1.2GHz = the phase-B floor); every bias-add/copy is
on DVE.  There is a single software-pipelined PE stream: the k/v
projection and v_ext transposes are fillers inside s-tile 0's loop, the
q-projections / output projections / denominator transposes are fillers
in later s-tiles, so the PE never idles long enough for the HAM clock
gate to re-throttle it to 1.2 GHz.  All input DMAs share the nc.sync
trigger queue in need-order: ctx0, x0, ctx1, ctx2, ctx3, x1, x2, x3.
"""

import numpy as np

B = 8
SQ = 2048
SK = 2048
D = 1024
H = 8
N_CORES = 8

_CACHE: dict = {}


def _build_nc():
    import concourse.bacc as bacc
    import concourse.mybir as mybir
    from concourse.bass import ds, ts
    from concourse.tile import TileContext

    F32 = mybir.dt.float32
    F16 = mybir.dt.float16
    BF16 = mybir.dt.bfloat16
    EXP = mybir.ActivationFunctionType.Exp

    nc = bacc.Bacc("TRN2", target_bir_lowering=False, debug=False)

    xT = nc.dram_tensor("xT", [D, SQ], F16, kind="ExternalInput").ap()
    ctxT = nc.dram_tensor("ctxT", [D, SK], F16, kind="ExternalInput").ap()
    wq_d = nc.dram_tensor("wq_l", [128, 832], F16, kind="ExternalInput").ap()
    wkv_d = nc.dram_tensor("wkv_l", [128, 832], F16, kind="ExternalInput").ap()
    bq_d = nc.dram_tensor("bq104", [104, 1], F32, kind="ExternalInput").ap()
    bkv_d = nc.dram_tensor("bkv104", [104, 1], F32, kind="ExternalInput").ap()
    wob_d = nc.dram_tensor("wob", [9, D], BF16, kind="ExternalInput").ap()
    id_d = nc.dram_tensor("ident33", [33, 33], BF16, kind="ExternalInput").ap()
    vc_d = nc.dram_tensor("vconst", [25, SK], BF16, kind="ExternalInput").ap()
    on_d = nc.dram_tensor("ones33", [33, 1], BF16, kind="ExternalInput").ap()
    y_d = nc.dram_tensor("y", [SQ, D], F16, kind="ExternalOutput").ap()

    with TileContext(nc) as tc:
        with tc.tile_pool(name="consts", bufs=1) as cp:
            wq_sb = cp.tile([128, 832], F16)
            wkv_sb = cp.tile([128, 832], F16)
            bq_sb = cp.tile([104, 1], F32)
            bkv_sb = cp.tile([104, 1], F32)
            wob_sb = cp.tile([9, D], BF16)
            id_sb = cp.tile([33, 33], BF16)
            ones_sb = cp.tile([33, 1], BF16)

            # persistent activations
            # k and q live replicated on partitions 32-39/64-71/96-103 so
            # consecutive scores matmuls hit different PE row groups and run
            # concurrently on different 32x32 sub-arrays.
            kT_sb = cp.tile([104, SK], F16)
            qT_sb = cp.tile([104, SQ], F16)
            vT1_sb = cp.tile([33, SK], BF16)  # v rows 0-7, ones rows 8 & 32
            vext_sb = cp.tile([128, 33 * 16], BF16)
            outU_sb = cp.tile([33, SQ], BF16)  # rows 0-7 outU, 8/32 denom
            rden_sb = cp.tile([128, 16], F32)

            # const DMAs on the gpsimd trigger queue (scalar queue stays
            # pure-exp, sync queue starts the big inputs at t=0), in
            # need-order: kv0 wants wkv, tr0 wants vconst+ident.
            # rows 8..32 of vT1 are constants (ones at 8 and 32, zeros
            # between) — engine writes need 32-aligned partition bases, so
            # fill them via DMA instead of memset.
            nc.gpsimd.dma_start(wkv_sb, wkv_d)
            nc.gpsimd.dma_start(bkv_sb, bkv_d)
            nc.gpsimd.dma_start(vT1_sb[8:33, :], vc_d)
            nc.gpsimd.dma_start(id_sb, id_d)
            nc.gpsimd.dma_start(wq_sb, wq_d)
            nc.gpsimd.dma_start(bq_sb, bq_d)
            nc.gpsimd.dma_start(wob_sb, wob_d)
            nc.gpsimd.dma_start(ones_sb, on_d)

            # PSUM budget (8 banks): kvq 1 + sc 2x2 + oe 1 = 6 outer,
            # + tr 1 during s-tile 0, then tr freed and yp 2 for s-tiles 1-3
            with tc.tile_pool(name="pin", bufs=4) as inp, \
                 tc.tile_pool(name="pxin", bufs=4) as xinp, \
                 tc.tile_pool(name="pkvq", bufs=1, space="PSUM") as pkvq, \
                 tc.tile_pool(name="psc", bufs=2, space="PSUM") as psc, \
                 tc.tile_pool(name="poe", bufs=1, space="PSUM") as poe, \
                 tc.tile_pool(name="pet", bufs=4) as etp, \
                 tc.tile_pool(name="pys", bufs=4) as ysp:
                # input DMAs all share the nc.sync trigger queue so they
                # transfer strictly in need-order.  The first ctx/x tiles
                # are split into D-row halves so kv0/qp0 start at half the
                # DMA latency.
                x_ts = [None] * 4
                ctx_ts = [None] * 4

                def emit_half(dst, src_t, tt, half):
                    t = dst.tile([128, 2048], F16, tag="h")
                    nc.sync.dma_start(
                        t.rearrange("p (c s) -> p c s", c=4),
                        src_t[ds(512 * half, 512), ts(tt, 512)].rearrange(
                            "(c p) s -> p c s", p=128),
                    )
                    return t

                def emit_xdma(st):
                    x_t = xinp.tile([128, 4096], F16, tag="xt")
                    nc.sync.dma_start(
                        x_t.rearrange("p (c s) -> p c s", c=8),
                        xT[:, ts(st, 512)].rearrange("(c p) s -> p c s", p=128),
                    )
                    x_ts[st] = (x_t, x_t, 0)

                def emit_ctxdma(tt):
                    ctx_t = inp.tile([128, 4096], F16, tag="ctx")
                    nc.sync.dma_start(
                        ctx_t.rearrange("p (c s) -> p c s", c=8),
                        ctxT[:, ts(tt, 512)].rearrange(
                            "(c p) s -> p c s", p=128),
                    )
                    ctx_ts[tt] = (ctx_t, ctx_t, 0)

                ctx_ts[0] = (emit_half(inp, ctxT, 0, 0),
                             emit_half(inp, ctxT, 0, 1), 1)
                x_ts[0] = (emit_half(xinp, xT, 0, 0),
                           emit_half(xinp, xT, 0, 1), 1)
                emit_ctxdma(1)
                emit_xdma(1)
                emit_ctxdma(2)
                emit_ctxdma(3)
                emit_xdma(2)
                emit_xdma(3)

                def slice_in(pair, dc):
                    lo, hi, is_split = pair
                    if is_split:
                        return (lo if dc < 4 else hi)[:, ts(dc % 4, 512)]
                    return lo[:, ts(dc, 512)]

                def emit_kv(tt):
                    kv_ps = pkvq.tile([104, 512], F32, tag="kvq")
                    for dc in range(8):
                        nc.tensor.matmul(
                            kv_ps, wkv_sb[:, ds(104 * dc, 104)],
                            slice_in(ctx_ts[tt], dc),
                            start=(dc == 0), stop=(dc == 7),
                        )
                    nc.vector.tensor_scalar_add(
                        vT1_sb[0:8, ts(tt, 512)], kv_ps[0:8, :],
                        bkv_sb[0:8, 0:1])
                    nc.vector.tensor_scalar_add(
                        kT_sb[32:64, ts(tt, 512)], kv_ps[32:64, :],
                        bkv_sb[32:64, 0:1])
                    nc.vector.tensor_scalar_add(
                        kT_sb[64:72, ts(tt, 512)], kv_ps[64:72, :],
                        bkv_sb[64:72, 0:1])

                def emit_tr(tt):
                    for c in range(4):
                        cc = 4 * tt + c
                        tr_ps = ptr.tile([128, 33], BF16, tag="tr")
                        nc.tensor.transpose(
                            tr_ps, vT1_sb[0:33, ts(cc, 128)], id_sb)
                        nc.vector.tensor_copy(
                            vext_sb[:, ds(33 * cc, 33)], tr_ps)

                def emit_qproj(st):
                    q_ps = pkvq.tile([104, 512], F32, tag="kvq")
                    for dc in range(8):
                        nc.tensor.matmul(
                            q_ps, wq_sb[:, ds(104 * dc, 104)],
                            slice_in(x_ts[st], dc),
                            start=(dc == 0), stop=(dc == 7),
                        )
                    nc.vector.tensor_scalar_add(
                        qT_sb[32:64, ts(st, 512)], q_ps[32:64, :],
                        bq_sb[32:64, 0:1])
                    nc.vector.tensor_scalar_add(
                        qT_sb[64:72, ts(st, 512)], q_ps[64:72, :],
                        bq_sb[64:72, 0:1])

                ets = {}

                def emit_scores(st, ee):
                    sc_ps = psc.tile([128, 1024], F32, tag="sc")
                    et = etp.tile([128, 1024], BF16, tag="et")
                    ets[(st, ee)] = et
                    for half in range(2):
                        tcn = 2 * ee + half
                        g = 32 * (1 + tcn % 2)   # row group alternates 32/64
                        nc.tensor.matmul(
                            sc_ps[:, ts(half, 512)],
                            kT_sb[g:g + 8, ts(tcn, 128)],
                            qT_sb[g:g + 8, ts(st, 512)],
                            start=True, stop=True,
                            tile_position=(g, 0),
                        )
                    nc.scalar.activation(et, sc_ps, EXP)

                def emit_oe(st, ee, oe_ps):
                    et = ets.pop((st, ee))
                    for half in range(2):
                        tcn = 2 * ee + half
                        nc.tensor.matmul(
                            oe_ps, vext_sb[:, ds(33 * tcn, 33)],
                            et[:, ts(half, 512)],
                            start=(tcn == 0), stop=(tcn == 15),
                        )

                def emit_dp(st):
                    # transpose denom row [1, 128] -> [128, 1] via K=1 MMs
                    dp = pyp.tile([128, 4], F32, tag="y")
                    for j in range(4):
                        nc.tensor.matmul(
                            dp[:, ds(j, 1)],
                            outU_sb[32:33, ds(512 * st + 128 * j, 128)],
                            ones_sb[32:33, 0:1],
                            start=True, stop=True,
                        )
                    nc.vector.reciprocal(rden_sb[:, ts(st, 4)], dp[:, 0:4])

                def emit_y(st, j):
                    y_sb = ysp.tile([128, 1024], F16, tag="ys")
                    for dh in range(2):
                        y_ps = pyp.tile([128, 512], F32, tag="y")
                        nc.tensor.matmul(
                            y_ps,
                            outU_sb[0:9, ds(512 * st + 128 * j, 128)],
                            wob_sb[:, ts(dh, 512)],
                            start=True, stop=True,
                        )
                        nc.vector.tensor_scalar_mul(
                            y_sb[:, ts(dh, 512)], y_ps,
                            rden_sb[:, ds(4 * st + j, 1)],
                        )
                    nc.gpsimd.dma_start(
                        y_d[ds(128 * (4 * st + j), 128), :], y_sb,
                    )

                # ---- staggered-wavefront PE pipeline ----
                # Each "slot" is one scores-pair (2 t-chunks x 512 s) + its
                # exp.  s-tiles run staggered (st0 a half-tile ahead of st1,
                # etc.) so ScalarE receives a dense exp stream from ~8us on
                # while k/v/q projections, transposes and output projections
                # slot in as PE fillers.
                SL = []
                for b in range(5):
                    for s in (b - 1, b):
                        if 0 <= s <= 3 and s <= b < s + 2:
                            SL.extend((s, 4 * (b - s) + k) for k in range(4))

                fillers = {
                    1: [lambda: emit_kv(1)],
                    3: [lambda: emit_tr(1), lambda: emit_kv(2)],
                    4: [lambda: emit_tr(2)],
                    5: [lambda: emit_kv(3)],
                    6: [lambda: emit_qproj(1), lambda: emit_tr(3)],
                    12: [lambda: emit_dp(0)],
                    13: [lambda: emit_y(0, 0)],
                    14: [lambda: emit_qproj(2)],
                    15: [lambda: emit_y(0, 1)],
                    16: [lambda: emit_y(0, 2)],
                    17: [lambda: emit_y(0, 3)],
                    19: [lambda: emit_dp(1)],
                    20: [lambda: emit_y(1, 0)],
                    21: [lambda: emit_y(1, 1)],
                    22: [lambda: emit_qproj(3)],
                    23: [lambda: emit_y(1, 2)],
                    24: [lambda: emit_y(1, 3)],
                    27: [lambda: emit_dp(2)],
                    28: [lambda: emit_y(2, 0)],
                    29: [lambda: emit_y(2, 1)],
                    30: [lambda: emit_y(2, 2)],
                    31: [lambda: emit_y(2, 3)],
                }

                oe_tiles = {}

                def do_slot(i):
                    for f in fillers.get(i, []):
                        f()
                    if i < len(SL):
                        emit_scores(*SL[i])
                    if i - 2 >= 0:
                        s, ee = SL[i - 2]
                        if ee == 0:
                            oe_tiles[s] = poe.tile(
                                [33, 512], F32, tag="oe", name=f"oe{s}")
                        emit_oe(s, ee, oe_tiles[s])
                        if ee == 7:
                            nc.vector.tensor_copy(
                                outU_sb[0:33, ts(s, 512)],
                                oe_tiles.pop(s)[0:33, :])

                with tc.tile_pool(name="ptr", bufs=1, space="PSUM") as ptr:
                    # Warm the PE's HAM clock gate during the otherwise-dead
                    # input-DMA window (~10us): a stream of tiny matmuls on
                    # the identity tile counts as sustained PE activity, so
                    # the first real projections run at 2.4 GHz instead of
                    # 1.2.  ~50 x ~70ns fits comfortably before ctx arrives.
                    warm_ps = ptr.tile([33, 512], F32, tag="tr")
                    for w in range(16):
                        nc.tensor.matmul(
                            warm_ps[0:33, 0:33], wkv_sb[:, 0:33],
                            wkv_sb[:, 0:33],
                            start=True, stop=True,
                        )
                    emit_kv(0)
                    emit_tr(0)
                    emit_qproj(0)
                    for i in range(7):
                        do_slot(i)
                with tc.tile_pool(name="pyp", bufs=2, space="PSUM") as pyp:
                    for i in range(7, len(SL) + 2):
                        do_slot(i)
                    emit_dp(3)
                    for j in range(4):
                        emit_y(3, j)

    nc.compile()
    return nc


def _get_nc():
    if "nc" not in _CACHE:
        _CACHE["nc"] = _build_nc()
    return _CACHE["nc"]


def _prep_params(Wq, bq, Wk, bk, Wv, bv, Wo, bo):
    from ml_dtypes import bfloat16
    f32, f16 = np.float32, np.float16
    Wq = np.asarray(Wq, f32)
    Wk = np.asarray(Wk, f32)
    Wv = np.asarray(Wv, f32)
    Wo = np.asarray(Wo, f32)
    # stationaries replicate k (and q) into output rows 32/64/96 for free:
    # per 128-row D-chunk, cols 0-7 = Wv, cols 32-39/64-71/96-103 = Wk (Wq)
    wq3 = np.zeros((8, 128, 104), f32)
    wkv = np.zeros((8, 128, 104), f32)
    wkv[:, :, 0:8] = Wv.reshape(8, 128, 8)
    for g in (32, 64, 96):
        wkv[:, :, g:g + 8] = Wk.reshape(8, 128, 8)
        wq3[:, :, g:g + 8] = Wq.reshape(8, 128, 8)
    wq_l = np.ascontiguousarray(
        wq3.transpose(1, 0, 2).reshape(128, 832)).astype(f16)
    wkv_l = np.ascontiguousarray(
        wkv.transpose(1, 0, 2).reshape(128, 832)).astype(f16)
    bkv = np.zeros((104, 1), f32)
    bkv[0:8, 0] = np.asarray(bv, f32)
    bq40 = np.zeros((104, 1), f32)
    for g in (32, 64, 96):
        bkv[g:g + 8, 0] = np.asarray(bk, f32)
        bq40[g:g + 8, 0] = np.asarray(bq, f32)
    wob = np.concatenate(
        [Wo, np.asarray(bo, f32)[None, :]], axis=0).astype(bfloat16)
    ident = np.eye(33, dtype=f32).astype(bfloat16)
    vconst = np.zeros((25, SK), f32)
    vconst[0, :] = 1.0   # vT1 row 8: denominator ones column
    vconst[24, :] = 1.0  # vT1 row 32: denominator copy for denomT matmul
    return {
        "wq_l": wq_l, "wkv_l": wkv_l,
        "bq104": bq40, "bkv104": bkv,
        "wob": np.ascontiguousarray(wob), "ident33": ident,
        "vconst": vconst.astype(bfloat16),
        "ones33": np.ones((33, 1), f32).astype(bfloat16),
    }


def make_in_maps(x, context, Wq, bq, Wk, bk, Wv, bv, Wo, bo):
    f16 = np.float16
    x = np.asarray(x, np.float32)
    context = np.asarray(context, np.float32)
    xT = np.ascontiguousarray(x.transpose(0, 2, 1)).astype(f16)  # [B, D, SQ]
    ctxT = np.ascontiguousarray(context.transpose(0, 2, 1)).astype(f16)
    params = _prep_params(Wq, bq, Wk, bk, Wv, bv, Wo, bo)
    return [
        {"xT": xT[b], "ctxT": ctxT[b], **params} for b in range(N_CORES)
    ]


def kernel(x, context, Wq, bq, Wk, bk, Wv, bv, Wo, bo):
    import concourse.bass_utils as bass_utils

    nc = _get_nc()
    in_maps = make_in_maps(x, context, Wq, bq, Wk, bk, Wv, bv, Wo, bo)
    res = bass_utils.run_bass_kernel_spmd(
        nc, in_maps, core_ids=list(range(N_CORES)))
    return np.stack(
        [res.results[b]["y"] for b in range(N_CORES)], axis=0
    ).astype(np.float32)


# revision 22
# speedup vs baseline: 1.0390x; 1.0390x over previous
"""Cross-attention Trainium2 Bass kernel (v3, 16-bit datapath, fused pipeline).

Math (per batch element b, one per NeuronCore):
    q = x Wq + bq            [Sq, 8]
    k = ctx Wk + bk          [Sk, 8]
    v = ctx Wv + bv          [Sk, 8]
    scores = q k^T           [Sq, Sk]
    w = softmax(scores)      (no max subtraction; scores are bounded ~|31|)
    out = w v                [Sq, 8]
    y = out Wo + bo          [Sq, 1024]

Dtypes: x/ctx/q/k fp16 (halves input DMA, keeps ~f32r-grade scores),
exp/v/out bf16 (exp needs e^31 range), y fp16 out (host upcasts).
Measured end-to-end rel err ~7e-3 vs f64 reference (gate 2e-2).

Layout (all contractions need the contracted dim on SBUF partitions, so
x/ctx are fed pre-transposed as xT/ctxT [1024, 2048]):
    kv_ps[40, t] = Wkv^T ctxT   (one fused pass; v rows 0-7, k rows 32-39)
    qT[40, s]    = Wq^T xT      (q on partitions 32-39, same PE row group
                                 as k so the scores matmul bases match)
    v_ext[t, 33] = PE-transpose of [vT; ones; 0...; ones] (ones rows 8, 32)
    E^T[t, s]    = exp(kT^T qT)          (scoresT via PE, exp via ScalarE)
    oe[33, s]    = v_ext^T E^T  accumulated over t-chunks:
        rows 0-7 = sum_t E v    rows 8, 32 = sum_t E  (softmax denominator)
    y[s, d]      = [out; den]^T [Wo; bo]  scaled by 1/den per-partition.

Scheduling: ScalarE (ACT) does nothing but the 32 exp instructions
(1 elem/cycle/lane @1.2GHz = the phase-B floor); every bias-add/copy is
on DVE.  There is a single software-pipelined PE stream: the k/v
projection and v_ext transposes are fillers inside s-tile 0's loop, the
q-projections / output projections / denominator transposes are fillers
in later s-tiles, so the PE never idles long enough for the HAM clock
gate to re-throttle it to 1.2 GHz.  All input DMAs share the nc.sync
trigger queue in need-order: ctx0, x0, ctx1, ctx2, ctx3, x1, x2, x3.
"""

import numpy as np

B = 8
SQ = 2048
SK = 2048
D = 1024
H = 8
N_CORES = 8

_CACHE: dict = {}


def _build_nc():
    import concourse.bacc as bacc
    import concourse.mybir as mybir
    from concourse.bass import ds, ts
    from concourse.tile import TileContext

    F32 = mybir.dt.float32
    F16 = mybir.dt.float16
    BF16 = mybir.dt.bfloat16
    EXP = mybir.ActivationFunctionType.Exp

    nc = bacc.Bacc("TRN2", target_bir_lowering=False, debug=False)

    xT = nc.dram_tensor("xT", [D, SQ], F16, kind="ExternalInput").ap()
    ctxT = nc.dram_tensor("ctxT", [D, SK], F16, kind="ExternalInput").ap()
    wq_d = nc.dram_tensor("wq_l", [128, 832], F16, kind="ExternalInput").ap()
    wkv_d = nc.dram_tensor("wkv_l", [128, 832], F16, kind="ExternalInput").ap()
    bq_d = nc.dram_tensor("bq104", [104, 1], F32, kind="ExternalInput").ap()
    bkv_d = nc.dram_tensor("bkv104", [104, 1], F32, kind="ExternalInput").ap()
    wob_d = nc.dram_tensor("wob", [9, D], BF16, kind="ExternalInput").ap()
    id_d = nc.dram_tensor("ident33", [33, 33], BF16, kind="ExternalInput").ap()
    vc_d = nc.dram_tensor("vconst", [25, SK], BF16, kind="ExternalInput").ap()
    on_d = nc.dram_tensor("ones33", [33, 1], BF16, kind="ExternalInput").ap()
    y_d = nc.dram_tensor("y", [SQ, D], F16, kind="ExternalOutput").ap()

    with TileContext(nc) as tc:
        with tc.tile_pool(name="consts", bufs=1) as cp:
            wq_sb = cp.tile([128, 832], F16)
            wkv_sb = cp.tile([128, 832], F16)
            bq_sb = cp.tile([104, 1], F32)
            bkv_sb = cp.tile([104, 1], F32)
            wob_sb = cp.tile([9, D], BF16)
            id_sb = cp.tile([33, 33], BF16)
            ones_sb = cp.tile([33, 1], BF16)

            # persistent activations
            # k and q live replicated on partitions 32-39/64-71/96-103 so
            # consecutive scores matmuls hit different PE row groups and run
            # concurrently on different 32x32 sub-arrays.
            kT_sb = cp.tile([104, SK], F16)
            qT_sb = cp.tile([104, SQ], F16)
            vT1_sb = cp.tile([33, SK], BF16)  # v rows 0-7, ones rows 8 & 32
            vext_sb = cp.tile([128, 33 * 16], BF16)
            outU_sb = cp.tile([33, SQ], BF16)  # rows 0-7 outU, 8/32 denom
            rden_sb = cp.tile([128, 16], F32)
            # DMA-independent warm-up source: memset by DVE at t=0 so the
            # PE's HAM clock gate can warm during the ~11us input-DMA
            # latency window.
            warm_src = cp.tile([128, 256], BF16)
            nc.vector.memset(warm_src, 0.03125)

            # const DMAs on the gpsimd trigger queue (scalar queue stays
            # pure-exp, sync queue starts the big inputs at t=0), in
            # need-order: kv0 wants wkv, tr0 wants vconst+ident.
            # rows 8..32 of vT1 are constants (ones at 8 and 32, zeros
            # between) — engine writes need 32-aligned partition bases, so
            # fill them via DMA instead of memset.
            nc.gpsimd.dma_start(wkv_sb, wkv_d)
            nc.gpsimd.dma_start(bkv_sb, bkv_d)
            nc.gpsimd.dma_start(vT1_sb[8:33, :], vc_d)
            nc.gpsimd.dma_start(id_sb, id_d)
            nc.gpsimd.dma_start(wq_sb, wq_d)
            nc.gpsimd.dma_start(bq_sb, bq_d)
            nc.gpsimd.dma_start(wob_sb, wob_d)
            nc.gpsimd.dma_start(ones_sb, on_d)

            # PSUM budget (8 banks): kvq 1 + sc 2x2 + oe 1 = 6 outer,
            # + tr 1 during s-tile 0, then tr freed and yp 2 for s-tiles 1-3
            with tc.tile_pool(name="pin", bufs=4) as inp, \
                 tc.tile_pool(name="pxin", bufs=4) as xinp, \
                 tc.tile_pool(name="pkvq", bufs=1, space="PSUM") as pkvq, \
                 tc.tile_pool(name="psc", bufs=2, space="PSUM") as psc, \
                 tc.tile_pool(name="poe", bufs=1, space="PSUM") as poe, \
                 tc.tile_pool(name="pet", bufs=4) as etp, \
                 tc.tile_pool(name="pys", bufs=4) as ysp:
                # input DMAs all share the nc.sync trigger queue so they
                # transfer strictly in need-order.  The first ctx/x tiles
                # are split into D-row halves so kv0/qp0 start at half the
                # DMA latency.
                x_ts = [None] * 4
                ctx_ts = [None] * 4

                def emit_half(dst, src_t, tt, half):
                    t = dst.tile([128, 2048], F16, tag="h")
                    nc.sync.dma_start(
                        t.rearrange("p (c s) -> p c s", c=4),
                        src_t[ds(512 * half, 512), ts(tt, 512)].rearrange(
                            "(c p) s -> p c s", p=128),
                    )
                    return t

                def emit_xdma(st):
                    x_t = xinp.tile([128, 4096], F16, tag="xt")
                    nc.sync.dma_start(
                        x_t.rearrange("p (c s) -> p c s", c=8),
                        xT[:, ts(st, 512)].rearrange("(c p) s -> p c s", p=128),
                    )
                    x_ts[st] = (x_t, x_t, 0)

                def emit_ctxdma(tt):
                    ctx_t = inp.tile([128, 4096], F16, tag="ctx")
                    nc.sync.dma_start(
                        ctx_t.rearrange("p (c s) -> p c s", c=8),
                        ctxT[:, ts(tt, 512)].rearrange(
                            "(c p) s -> p c s", p=128),
                    )
                    ctx_ts[tt] = (ctx_t, ctx_t, 0)

                ctx_ts[0] = (emit_half(inp, ctxT, 0, 0),
                             emit_half(inp, ctxT, 0, 1), 1)
                x_ts[0] = (emit_half(xinp, xT, 0, 0),
                           emit_half(xinp, xT, 0, 1), 1)
                emit_ctxdma(1)
                emit_xdma(1)
                emit_ctxdma(2)
                emit_ctxdma(3)
                emit_xdma(2)
                emit_xdma(3)

                def slice_in(pair, dc):
                    lo, hi, is_split = pair
                    if is_split:
                        return (lo if dc < 4 else hi)[:, ts(dc % 4, 512)]
                    return lo[:, ts(dc, 512)]

                def emit_kv(tt):
                    kv_ps = pkvq.tile([104, 512], F32, tag="kvq")
                    for dc in range(8):
                        nc.tensor.matmul(
                            kv_ps, wkv_sb[:, ds(104 * dc, 104)],
                            slice_in(ctx_ts[tt], dc),
                            start=(dc == 0), stop=(dc == 7),
                        )
                    nc.vector.tensor_scalar_add(
                        vT1_sb[0:8, ts(tt, 512)], kv_ps[0:8, :],
                        bkv_sb[0:8, 0:1])
                    nc.vector.tensor_scalar_add(
                        kT_sb[32:64, ts(tt, 512)], kv_ps[32:64, :],
                        bkv_sb[32:64, 0:1])
                    nc.vector.tensor_scalar_add(
                        kT_sb[64:72, ts(tt, 512)], kv_ps[64:72, :],
                        bkv_sb[64:72, 0:1])

                def emit_tr(tt):
                    for c in range(4):
                        cc = 4 * tt + c
                        tr_ps = ptr.tile([128, 33], BF16, tag="tr")
                        nc.tensor.transpose(
                            tr_ps, vT1_sb[0:33, ts(cc, 128)], id_sb)
                        nc.vector.tensor_copy(
                            vext_sb[:, ds(33 * cc, 33)], tr_ps)

                def emit_qproj(st):
                    q_ps = pkvq.tile([104, 512], F32, tag="kvq")
                    for dc in range(8):
                        nc.tensor.matmul(
                            q_ps, wq_sb[:, ds(104 * dc, 104)],
                            slice_in(x_ts[st], dc),
                            start=(dc == 0), stop=(dc == 7),
                        )
                    nc.vector.tensor_scalar_add(
                        qT_sb[32:64, ts(st, 512)], q_ps[32:64, :],
                        bq_sb[32:64, 0:1])
                    nc.vector.tensor_scalar_add(
                        qT_sb[64:72, ts(st, 512)], q_ps[64:72, :],
                        bq_sb[64:72, 0:1])

                ets = {}

                def emit_scores(st, ee):
                    sc_ps = psc.tile([128, 1024], F32, tag="sc")
                    et = etp.tile([128, 1024], BF16, tag="et")
                    ets[(st, ee)] = et
                    for half in range(2):
                        tcn = 2 * ee + half
                        g = 32 * (1 + tcn % 2)   # row group alternates 32/64
                        nc.tensor.matmul(
                            sc_ps[:, ts(half, 512)],
                            kT_sb[g:g + 8, ts(tcn, 128)],
                            qT_sb[g:g + 8, ts(st, 512)],
                            start=True, stop=True,
                            tile_position=(g, 0),
                        )
                    nc.scalar.activation(et, sc_ps, EXP)

                def emit_oe(st, ee, oe_ps):
                    et = ets.pop((st, ee))
                    for half in range(2):
                        tcn = 2 * ee + half
                        nc.tensor.matmul(
                            oe_ps, vext_sb[:, ds(33 * tcn, 33)],
                            et[:, ts(half, 512)],
                            start=(tcn == 0), stop=(tcn == 15),
                        )

                def emit_dp(st):
                    # transpose denom row [1, 128] -> [128, 1] via K=1 MMs
                    dp = pyp.tile([128, 4], F32, tag="y")
                    for j in range(4):
                        nc.tensor.matmul(
                            dp[:, ds(j, 1)],
                            outU_sb[32:33, ds(512 * st + 128 * j, 128)],
                            ones_sb[32:33, 0:1],
                            start=True, stop=True,
                        )
                    nc.vector.reciprocal(rden_sb[:, ts(st, 4)], dp[:, 0:4])

                def emit_y(st, j):
                    y_sb = ysp.tile([128, 1024], F16, tag="ys")
                    for dh in range(2):
                        y_ps = pyp.tile([128, 512], F32, tag="y")
                        nc.tensor.matmul(
                            y_ps,
                            outU_sb[0:9, ds(512 * st + 128 * j, 128)],
                            wob_sb[:, ts(dh, 512)],
                            start=True, stop=True,
                        )
                        nc.vector.tensor_scalar_mul(
                            y_sb[:, ts(dh, 512)], y_ps,
                            rden_sb[:, ds(4 * st + j, 1)],
                        )
                    nc.gpsimd.dma_start(
                        y_d[ds(128 * (4 * st + j), 128), :], y_sb,
                    )

                # ---- staggered-wavefront PE pipeline ----
                # Each "slot" is one scores-pair (2 t-chunks x 512 s) + its
                # exp.  s-tiles run staggered (st0 a half-tile ahead of st1,
                # etc.) so ScalarE receives a dense exp stream from ~8us on
                # while k/v/q projections, transposes and output projections
                # slot in as PE fillers.
                SL = []
                for b in range(5):
                    for s in (b - 1, b):
                        if 0 <= s <= 3 and s <= b < s + 2:
                            SL.extend((s, 4 * (b - s) + k) for k in range(4))

                fillers = {
                    1: [lambda: emit_kv(1)],
                    3: [lambda: emit_tr(1), lambda: emit_kv(2)],
                    4: [lambda: emit_tr(2)],
                    5: [lambda: emit_kv(3)],
                    6: [lambda: emit_qproj(1), lambda: emit_tr(3)],
                    12: [lambda: emit_dp(0)],
                    13: [lambda: emit_y(0, 0)],
                    14: [lambda: emit_qproj(2)],
                    15: [lambda: emit_y(0, 1)],
                    16: [lambda: emit_y(0, 2)],
                    17: [lambda: emit_y(0, 3)],
                    19: [lambda: emit_dp(1)],
                    20: [lambda: emit_y(1, 0)],
                    21: [lambda: emit_y(1, 1)],
                    22: [lambda: emit_qproj(3)],
                    23: [lambda: emit_y(1, 2)],
                    24: [lambda: emit_y(1, 3)],
                    27: [lambda: emit_dp(2)],
                    28: [lambda: emit_y(2, 0)],
                    29: [lambda: emit_y(2, 1)],
                    30: [lambda: emit_y(2, 2)],
                    31: [lambda: emit_y(2, 3)],
                }

                oe_tiles = {}

                def do_slot(i):
                    for f in fillers.get(i, []):
                        f()
                    if i < len(SL):
                        emit_scores(*SL[i])
                    if i - 2 >= 0:
                        s, ee = SL[i - 2]
                        if ee == 0:
                            oe_tiles[s] = poe.tile(
                                [33, 512], F32, tag="oe", name=f"oe{s}")
                        emit_oe(s, ee, oe_tiles[s])
                        if ee == 7:
                            nc.vector.tensor_copy(
                                outU_sb[0:33, ts(s, 512)],
                                oe_tiles.pop(s)[0:33, :])

                with tc.tile_pool(name="ptr", bufs=1, space="PSUM") as ptr:
                    # Warm the PE's HAM clock gate during the otherwise-dead
                    # input-DMA window (~11us): a stream of matmuls on the
                    # memset tile keeps the PE busy from ~0.6us so the first
                    # real projections run at 2.4 GHz instead of 1.2.  Sized
                    # to end just before ctx0 lands.
                    warm_ps = ptr.tile([33, 512], F32, tag="tr")
                    for w in range(30):
                        nc.tensor.matmul(
                            warm_ps[0:33, 0:256], warm_src[:, 0:33],
                            warm_src,
                            start=True, stop=True,
                        )
                    emit_kv(0)
                    emit_qproj(0)
                    do_slot(0)
                    do_slot(1)
                    emit_tr(0)
                    for i in range(2, 7):
                        do_slot(i)
                with tc.tile_pool(name="pyp", bufs=2, space="PSUM") as pyp:
                    for i in range(7, len(SL) + 2):
                        do_slot(i)
                    emit_dp(3)
                    for j in range(4):
                        emit_y(3, j)

    nc.compile()
    return nc


def _get_nc():
    if "nc" not in _CACHE:
        _CACHE["nc"] = _build_nc()
    return _CACHE["nc"]


def _prep_params(Wq, bq, Wk, bk, Wv, bv, Wo, bo):
    from ml_dtypes import bfloat16
    f32, f16 = np.float32, np.float16
    Wq = np.asarray(Wq, f32)
    Wk = np.asarray(Wk, f32)
    Wv = np.asarray(Wv, f32)
    Wo = np.asarray(Wo, f32)
    # stationaries replicate k (and q) into output rows 32/64/96 for free:
    # per 128-row D-chunk, cols 0-7 = Wv, cols 32-39/64-71/96-103 = Wk (Wq)
    wq3 = np.zeros((8, 128, 104), f32)
    wkv = np.zeros((8, 128, 104), f32)
    wkv[:, :, 0:8] = Wv.reshape(8, 128, 8)
    for g in (32, 64, 96):
        wkv[:, :, g:g + 8] = Wk.reshape(8, 128, 8)
        wq3[:, :, g:g + 8] = Wq.reshape(8, 128, 8)
    wq_l = np.ascontiguousarray(
        wq3.transpose(1, 0, 2).reshape(128, 832)).astype(f16)
    wkv_l = np.ascontiguousarray(
        wkv.transpose(1, 0, 2).reshape(128, 832)).astype(f16)
    bkv = np.zeros((104, 1), f32)
    bkv[0:8, 0] = np.asarray(bv, f32)
    bq40 = np.zeros((104, 1), f32)
    for g in (32, 64, 96):
        bkv[g:g + 8, 0] = np.asarray(bk, f32)
        bq40[g:g + 8, 0] = np.asarray(bq, f32)
    wob = np.concatenate(
        [Wo, np.asarray(bo, f32)[None, :]], axis=0).astype(bfloat16)
    ident = np.eye(33, dtype=f32).astype(bfloat16)
    vconst = np.zeros((25, SK), f32)
    vconst[0, :] = 1.0   # vT1 row 8: denominator ones column
    vconst[24, :] = 1.0  # vT1 row 32: denominator copy for denomT matmul
    return {
        "wq_l": wq_l, "wkv_l": wkv_l,
        "bq104": bq40, "bkv104": bkv,
        "wob": np.ascontiguousarray(wob), "ident33": ident,
        "vconst": vconst.astype(bfloat16),
        "ones33": np.ones((33, 1), f32).astype(bfloat16),
    }


def make_in_maps(x, context, Wq, bq, Wk, bk, Wv, bv, Wo, bo):
    f16 = np.float16
    x = np.asarray(x, np.float32)
    context = np.asarray(context, np.float32)
    xT = np.ascontiguousarray(x.transpose(0, 2, 1)).astype(f16)  # [B, D, SQ]
    ctxT = np.ascontiguousarray(context.transpose(0, 2, 1)).astype(f16)
    params = _prep_params(Wq, bq, Wk, bk, Wv, bv, Wo, bo)
    return [
        {"xT": xT[b], "ctxT": ctxT[b], **params} for b in range(N_CORES)
    ]


def kernel(x, context, Wq, bq, Wk, bk, Wv, bv, Wo, bo):
    import concourse.bass_utils as bass_utils

    nc = _get_nc()
    in_maps = make_in_maps(x, context, Wq, bq, Wk, bk, Wv, bv, Wo, bo)
    res = bass_utils.run_bass_kernel_spmd(
        nc, in_maps, core_ids=list(range(N_CORES)))
    return np.stack(
        [res.results[b]["y"] for b in range(N_CORES)], axis=0
    ).astype(np.float32)


# revision 23
# speedup vs baseline: 1.0657x; 1.0257x over previous
"""Cross-attention Trainium2 Bass kernel (v3, 16-bit datapath, fused pipeline).

Math (per batch element b, one per NeuronCore):
    q = x Wq + bq            [Sq, 8]
    k = ctx Wk + bk          [Sk, 8]
    v = ctx Wv + bv          [Sk, 8]
    scores = q k^T           [Sq, Sk]
    w = softmax(scores)      (no max subtraction; scores are bounded ~|31|)
    out = w v                [Sq, 8]
    y = out Wo + bo          [Sq, 1024]

Dtypes: x/ctx/q/k fp16 (halves input DMA, keeps ~f32r-grade scores),
exp/v/out bf16 (exp needs e^31 range), y fp16 out (host upcasts).
Measured end-to-end rel err ~7e-3 vs f64 reference (gate 2e-2).

Layout (all contractions need the contracted dim on SBUF partitions, so
x/ctx are fed pre-transposed as xT/ctxT [1024, 2048]):
    kv_ps[40, t] = Wkv^T ctxT   (one fused pass; v rows 0-7, k rows 32-39)
    qT[40, s]    = Wq^T xT      (q on partitions 32-39, same PE row group
                                 as k so the scores matmul bases match)
    v_ext[t, 33] = PE-transpose of [vT; ones; 0...; ones] (ones rows 8, 32)
    E^T[t, s]    = exp(kT^T qT)          (scoresT via PE, exp via ScalarE)
    oe[33, s]    = v_ext^T E^T  accumulated over t-chunks:
        rows 0-7 = sum_t E v    rows 8, 32 = sum_t E  (softmax denominator)
    y[s, d]      = [out; den]^T [Wo; bo]  scaled by 1/den per-partition.

Scheduling: ScalarE (ACT) does nothing but the 32 exp instructions
(1 elem/cycle/lane @1.2GHz = the phase-B floor); every bias-add/copy is
on DVE.  There is a single software-pipelined PE stream: the k/v
projection and v_ext transposes are fillers inside s-tile 0's loop, the
q-projections / output projections / denominator transposes are fillers
in later s-tiles, so the PE never idles long enough for the HAM clock
gate to re-throttle it to 1.2 GHz.  All input DMAs share the nc.sync
trigger queue in need-order: ctx0, x0, ctx1, ctx2, ctx3, x1, x2, x3.
"""

import numpy as np

B = 8
SQ = 2048
SK = 2048
D = 1024
H = 8
N_CORES = 8

_CACHE: dict = {}


def _build_nc():
    import concourse.bacc as bacc
    import concourse.mybir as mybir
    from concourse.bass import ds, ts
    from concourse.tile import TileContext

    F32 = mybir.dt.float32
    F16 = mybir.dt.float16
    BF16 = mybir.dt.bfloat16
    EXP = mybir.ActivationFunctionType.Exp

    nc = bacc.Bacc("TRN2", target_bir_lowering=False, debug=False)

    xT = nc.dram_tensor("xT", [D, SQ], F16, kind="ExternalInput").ap()
    ctxT = nc.dram_tensor("ctxT", [D, SK], F16, kind="ExternalInput").ap()
    # prologue-critical first blocks, host-preformatted to SBUF layout so
    # their DMAs are simple contiguous copies (fast static-descriptor path)
    c0a_d = nc.dram_tensor("c0a", [128, 2048], F16, kind="ExternalInput").ap()
    c0b_d = nc.dram_tensor("c0b", [128, 2048], F16, kind="ExternalInput").ap()
    x0a_d = nc.dram_tensor("x0a", [128, 2048], F16, kind="ExternalInput").ap()
    x0b_d = nc.dram_tensor("x0b", [128, 2048], F16, kind="ExternalInput").ap()
    wq_d = nc.dram_tensor("wq_l", [128, 832], F16, kind="ExternalInput").ap()
    wkv_d = nc.dram_tensor("wkv_l", [128, 832], F16, kind="ExternalInput").ap()
    bq_d = nc.dram_tensor("bq104", [104, 1], F32, kind="ExternalInput").ap()
    bkv_d = nc.dram_tensor("bkv104", [104, 1], F32, kind="ExternalInput").ap()
    wob_d = nc.dram_tensor("wob", [9, D], BF16, kind="ExternalInput").ap()
    id_d = nc.dram_tensor("ident33", [33, 33], BF16, kind="ExternalInput").ap()
    vc_d = nc.dram_tensor("vconst", [25, SK], BF16, kind="ExternalInput").ap()
    on_d = nc.dram_tensor("ones33", [33, 1], BF16, kind="ExternalInput").ap()
    y_d = nc.dram_tensor("y", [SQ, D], F16, kind="ExternalOutput").ap()

    with TileContext(nc) as tc:
        with tc.tile_pool(name="consts", bufs=1) as cp:
            wq_sb = cp.tile([128, 832], F16)
            wkv_sb = cp.tile([128, 832], F16)
            bq_sb = cp.tile([104, 1], F32)
            bkv_sb = cp.tile([104, 1], F32)
            wob_sb = cp.tile([9, D], BF16)
            id_sb = cp.tile([33, 33], BF16)
            ones_sb = cp.tile([33, 1], BF16)

            # persistent activations
            # k and q live replicated on partitions 32-39/64-71/96-103 so
            # consecutive scores matmuls hit different PE row groups and run
            # concurrently on different 32x32 sub-arrays.
            kT_sb = cp.tile([104, SK], F16)
            qT_sb = cp.tile([104, SQ], F16)
            vT1_sb = cp.tile([33, SK], BF16)  # v rows 0-7, ones rows 8 & 32
            vext_sb = cp.tile([128, 33 * 16], BF16)
            outU_sb = cp.tile([33, SQ], BF16)  # rows 0-7 outU, 8/32 denom
            rden_sb = cp.tile([128, 16], F32)
            # DMA-independent warm-up source: memset by DVE at t=0 so the
            # PE's HAM clock gate can warm during the ~11us input-DMA
            # latency window.
            warm_src = cp.tile([128, 256], BF16)
            nc.vector.memset(warm_src, 0.03125)

            # const DMAs on the gpsimd trigger queue (scalar queue stays
            # pure-exp, sync queue starts the big inputs at t=0), in
            # need-order: kv0 wants wkv, tr0 wants vconst+ident.
            # rows 8..32 of vT1 are constants (ones at 8 and 32, zeros
            # between) — engine writes need 32-aligned partition bases, so
            # fill them via DMA instead of memset.
            nc.gpsimd.dma_start(wkv_sb, wkv_d)
            nc.gpsimd.dma_start(bkv_sb, bkv_d)
            nc.gpsimd.dma_start(vT1_sb[8:33, :], vc_d)
            nc.gpsimd.dma_start(id_sb, id_d)
            nc.gpsimd.dma_start(wq_sb, wq_d)
            nc.gpsimd.dma_start(bq_sb, bq_d)
            nc.gpsimd.dma_start(wob_sb, wob_d)
            nc.gpsimd.dma_start(ones_sb, on_d)

            # PSUM budget (8 banks): kvq 1 + sc 2x2 + oe 1 = 6 outer,
            # + tr 1 during s-tile 0, then tr freed and yp 2 for s-tiles 1-3
            with tc.tile_pool(name="pin", bufs=4) as inp, \
                 tc.tile_pool(name="pxin", bufs=4) as xinp, \
                 tc.tile_pool(name="pkvq", bufs=1, space="PSUM") as pkvq, \
                 tc.tile_pool(name="psc", bufs=2, space="PSUM") as psc, \
                 tc.tile_pool(name="poe", bufs=1, space="PSUM") as poe, \
                 tc.tile_pool(name="pet", bufs=4) as etp, \
                 tc.tile_pool(name="pys", bufs=4) as ysp:
                # input DMAs all share the nc.sync trigger queue so they
                # transfer strictly in need-order.  The first ctx/x tiles
                # are split into D-row halves so kv0/qp0 start at half the
                # DMA latency.
                x_ts = [None] * 4
                ctx_ts = [None] * 4

                def emit_pre(dst, pre_d):
                    t = dst.tile([128, 2048], F16, tag="h")
                    nc.sync.dma_start(t, pre_d)
                    return t

                def emit_xdma(st):
                    x_t = xinp.tile([128, 4096], F16, tag="xt")
                    nc.sync.dma_start(
                        x_t.rearrange("p (c s) -> p c s", c=8),
                        xT[:, ts(st, 512)].rearrange("(c p) s -> p c s", p=128),
                    )
                    x_ts[st] = (x_t, x_t, 0)

                def emit_ctxdma(tt):
                    ctx_t = inp.tile([128, 4096], F16, tag="ctx")
                    nc.sync.dma_start(
                        ctx_t.rearrange("p (c s) -> p c s", c=8),
                        ctxT[:, ts(tt, 512)].rearrange(
                            "(c p) s -> p c s", p=128),
                    )
                    ctx_ts[tt] = (ctx_t, ctx_t, 0)

                ctx_ts[0] = (emit_pre(inp, c0a_d),
                             emit_pre(inp, c0b_d), 1)
                x_ts[0] = (emit_pre(xinp, x0a_d),
                           emit_pre(xinp, x0b_d), 1)
                emit_ctxdma(1)
                emit_xdma(1)
                emit_ctxdma(2)
                emit_ctxdma(3)
                emit_xdma(2)
                emit_xdma(3)

                def slice_in(pair, dc):
                    lo, hi, is_split = pair
                    if is_split:
                        return (lo if dc < 4 else hi)[:, ts(dc % 4, 512)]
                    return lo[:, ts(dc, 512)]

                def emit_kv(tt):
                    kv_ps = pkvq.tile([104, 512], F32, tag="kvq")
                    for dc in range(8):
                        nc.tensor.matmul(
                            kv_ps, wkv_sb[:, ds(104 * dc, 104)],
                            slice_in(ctx_ts[tt], dc),
                            start=(dc == 0), stop=(dc == 7),
                        )
                    nc.vector.tensor_scalar_add(
                        vT1_sb[0:8, ts(tt, 512)], kv_ps[0:8, :],
                        bkv_sb[0:8, 0:1])
                    nc.vector.tensor_scalar_add(
                        kT_sb[32:64, ts(tt, 512)], kv_ps[32:64, :],
                        bkv_sb[32:64, 0:1])
                    nc.vector.tensor_scalar_add(
                        kT_sb[64:72, ts(tt, 512)], kv_ps[64:72, :],
                        bkv_sb[64:72, 0:1])

                def emit_tr(tt):
                    for c in range(4):
                        cc = 4 * tt + c
                        tr_ps = ptr.tile([128, 33], BF16, tag="tr")
                        nc.tensor.transpose(
                            tr_ps, vT1_sb[0:33, ts(cc, 128)], id_sb)
                        nc.vector.tensor_copy(
                            vext_sb[:, ds(33 * cc, 33)], tr_ps)

                def emit_qproj(st):
                    q_ps = pkvq.tile([104, 512], F32, tag="kvq")
                    for dc in range(8):
                        nc.tensor.matmul(
                            q_ps, wq_sb[:, ds(104 * dc, 104)],
                            slice_in(x_ts[st], dc),
                            start=(dc == 0), stop=(dc == 7),
                        )
                    nc.vector.tensor_scalar_add(
                        qT_sb[32:64, ts(st, 512)], q_ps[32:64, :],
                        bq_sb[32:64, 0:1])
                    nc.vector.tensor_scalar_add(
                        qT_sb[64:72, ts(st, 512)], q_ps[64:72, :],
                        bq_sb[64:72, 0:1])

                ets = {}

                def emit_scores(st, ee):
                    sc_ps = psc.tile([128, 1024], F32, tag="sc")
                    et = etp.tile([128, 1024], BF16, tag="et")
                    ets[(st, ee)] = et
                    for half in range(2):
                        tcn = 2 * ee + half
                        g = 32 * (1 + tcn % 2)   # row group alternates 32/64
                        nc.tensor.matmul(
                            sc_ps[:, ts(half, 512)],
                            kT_sb[g:g + 8, ts(tcn, 128)],
                            qT_sb[g:g + 8, ts(st, 512)],
                            start=True, stop=True,
                            tile_position=(g, 0),
                        )
                    nc.scalar.activation(et, sc_ps, EXP)

                def emit_oe(st, ee, oe_ps):
                    et = ets.pop((st, ee))
                    for half in range(2):
                        tcn = 2 * ee + half
                        nc.tensor.matmul(
                            oe_ps, vext_sb[:, ds(33 * tcn, 33)],
                            et[:, ts(half, 512)],
                            start=(tcn == 0), stop=(tcn == 15),
                        )

                def emit_dp(st):
                    # transpose denom row [1, 128] -> [128, 1] via K=1 MMs
                    dp = pyp.tile([128, 4], F32, tag="y")
                    for j in range(4):
                        nc.tensor.matmul(
                            dp[:, ds(j, 1)],
                            outU_sb[32:33, ds(512 * st + 128 * j, 128)],
                            ones_sb[32:33, 0:1],
                            start=True, stop=True,
                        )
                    nc.vector.reciprocal(rden_sb[:, ts(st, 4)], dp[:, 0:4])

                def emit_y(st, j):
                    y_sb = ysp.tile([128, 1024], F16, tag="ys")
                    for dh in range(2):
                        y_ps = pyp.tile([128, 512], F32, tag="y")
                        nc.tensor.matmul(
                            y_ps,
                            outU_sb[0:9, ds(512 * st + 128 * j, 128)],
                            wob_sb[:, ts(dh, 512)],
                            start=True, stop=True,
                        )
                        nc.vector.tensor_scalar_mul(
                            y_sb[:, ts(dh, 512)], y_ps,
                            rden_sb[:, ds(4 * st + j, 1)],
                        )
                    nc.gpsimd.dma_start(
                        y_d[ds(128 * (4 * st + j), 128), :], y_sb,
                    )

                # ---- staggered-wavefront PE pipeline ----
                # Each "slot" is one scores-pair (2 t-chunks x 512 s) + its
                # exp.  s-tiles run staggered (st0 a half-tile ahead of st1,
                # etc.) so ScalarE receives a dense exp stream from ~8us on
                # while k/v/q projections, transposes and output projections
                # slot in as PE fillers.
                SL = []
                for b in range(5):
                    for s in (b - 1, b):
                        if 0 <= s <= 3 and s <= b < s + 2:
                            SL.extend((s, 4 * (b - s) + k) for k in range(4))

                fillers = {
                    1: [lambda: emit_kv(1)],
                    3: [lambda: emit_tr(1), lambda: emit_kv(2)],
                    4: [lambda: emit_tr(2)],
                    5: [lambda: emit_kv(3)],
                    6: [lambda: emit_qproj(1), lambda: emit_tr(3)],
                    12: [lambda: emit_dp(0)],
                    13: [lambda: emit_y(0, 0)],
                    14: [lambda: emit_qproj(2)],
                    15: [lambda: emit_y(0, 1)],
                    16: [lambda: emit_y(0, 2)],
                    17: [lambda: emit_y(0, 3)],
                    19: [lambda: emit_dp(1)],
                    20: [lambda: emit_y(1, 0)],
                    21: [lambda: emit_y(1, 1)],
                    22: [lambda: emit_qproj(3)],
                    23: [lambda: emit_y(1, 2)],
                    24: [lambda: emit_y(1, 3)],
                    27: [lambda: emit_dp(2)],
                    28: [lambda: emit_y(2, 0)],
                    29: [lambda: emit_y(2, 1)],
                    30: [lambda: emit_y(2, 2)],
                    31: [lambda: emit_y(2, 3)],
                }

                oe_tiles = {}

                def do_slot(i):
                    for f in fillers.get(i, []):
                        f()
                    if i < len(SL):
                        emit_scores(*SL[i])
                    if i - 2 >= 0:
                        s, ee = SL[i - 2]
                        if ee == 0:
                            oe_tiles[s] = poe.tile(
                                [33, 512], F32, tag="oe", name=f"oe{s}")
                        emit_oe(s, ee, oe_tiles[s])
                        if ee == 7:
                            nc.vector.tensor_copy(
                                outU_sb[0:33, ts(s, 512)],
                                oe_tiles.pop(s)[0:33, :])

                with tc.tile_pool(name="ptr", bufs=1, space="PSUM") as ptr:
                    # Warm the PE's HAM clock gate during the otherwise-dead
                    # input-DMA window (~11us): a stream of matmuls on the
                    # memset tile keeps the PE busy from ~0.6us so the first
                    # real projections run at 2.4 GHz instead of 1.2.  Sized
                    # to end just before ctx0 lands.
                    warm_ps = ptr.tile([33, 512], F32, tag="tr")
                    for w in range(30):
                        nc.tensor.matmul(
                            warm_ps[0:33, 0:256], warm_src[:, 0:33],
                            warm_src,
                            start=True, stop=True,
                        )
                    emit_kv(0)
                    emit_qproj(0)
                    do_slot(0)
                    do_slot(1)
                    emit_tr(0)
                    for i in range(2, 7):
                        do_slot(i)
                with tc.tile_pool(name="pyp", bufs=2, space="PSUM") as pyp:
                    for i in range(7, len(SL) + 2):
                        do_slot(i)
                    emit_dp(3)
                    for j in range(4):
                        emit_y(3, j)

    nc.compile()
    return nc


def _get_nc():
    if "nc" not in _CACHE:
        _CACHE["nc"] = _build_nc()
    return _CACHE["nc"]


def _prep_params(Wq, bq, Wk, bk, Wv, bv, Wo, bo):
    from ml_dtypes import bfloat16
    f32, f16 = np.float32, np.float16
    Wq = np.asarray(Wq, f32)
    Wk = np.asarray(Wk, f32)
    Wv = np.asarray(Wv, f32)
    Wo = np.asarray(Wo, f32)
    # stationaries replicate k (and q) into output rows 32/64/96 for free:
    # per 128-row D-chunk, cols 0-7 = Wv, cols 32-39/64-71/96-103 = Wk (Wq)
    wq3 = np.zeros((8, 128, 104), f32)
    wkv = np.zeros((8, 128, 104), f32)
    wkv[:, :, 0:8] = Wv.reshape(8, 128, 8)
    for g in (32, 64, 96):
        wkv[:, :, g:g + 8] = Wk.reshape(8, 128, 8)
        wq3[:, :, g:g + 8] = Wq.reshape(8, 128, 8)
    wq_l = np.ascontiguousarray(
        wq3.transpose(1, 0, 2).reshape(128, 832)).astype(f16)
    wkv_l = np.ascontiguousarray(
        wkv.transpose(1, 0, 2).reshape(128, 832)).astype(f16)
    bkv = np.zeros((104, 1), f32)
    bkv[0:8, 0] = np.asarray(bv, f32)
    bq40 = np.zeros((104, 1), f32)
    for g in (32, 64, 96):
        bkv[g:g + 8, 0] = np.asarray(bk, f32)
        bq40[g:g + 8, 0] = np.asarray(bq, f32)
    wob = np.concatenate(
        [Wo, np.asarray(bo, f32)[None, :]], axis=0).astype(bfloat16)
    ident = np.eye(33, dtype=f32).astype(bfloat16)
    vconst = np.zeros((25, SK), f32)
    vconst[0, :] = 1.0   # vT1 row 8: denominator ones column
    vconst[24, :] = 1.0  # vT1 row 32: denominator copy for denomT matmul
    return {
        "wq_l": wq_l, "wkv_l": wkv_l,
        "bq104": bq40, "bkv104": bkv,
        "wob": np.ascontiguousarray(wob), "ident33": ident,
        "vconst": vconst.astype(bfloat16),
        "ones33": np.ones((33, 1), f32).astype(bfloat16),
    }


def make_in_maps(x, context, Wq, bq, Wk, bk, Wv, bv, Wo, bo):
    f16 = np.float16
    x = np.asarray(x, np.float32)
    context = np.asarray(context, np.float32)
    xT = np.ascontiguousarray(x.transpose(0, 2, 1)).astype(f16)  # [B, D, SQ]
    ctxT = np.ascontiguousarray(context.transpose(0, 2, 1)).astype(f16)
    params = _prep_params(Wq, bq, Wk, bk, Wv, bv, Wo, bo)

    def preblock(mat, half):
        # D-rows [512h, 512h+512) x s-cols [0, 512) in SBUF [128, (c s)]
        blk = mat[512 * half:512 * half + 512, 0:512]
        return np.ascontiguousarray(
            blk.reshape(4, 128, 512).transpose(1, 0, 2).reshape(128, 2048))

    return [
        {"xT": xT[b], "ctxT": ctxT[b],
         "c0a": preblock(ctxT[b], 0), "c0b": preblock(ctxT[b], 1),
         "x0a": preblock(xT[b], 0), "x0b": preblock(xT[b], 1),
         **params} for b in range(N_CORES)
    ]


def kernel(x, context, Wq, bq, Wk, bk, Wv, bv, Wo, bo):
    import concourse.bass_utils as bass_utils

    nc = _get_nc()
    in_maps = make_in_maps(x, context, Wq, bq, Wk, bk, Wv, bv, Wo, bo)
    res = bass_utils.run_bass_kernel_spmd(
        nc, in_maps, core_ids=list(range(N_CORES)))
    return np.stack(
        [res.results[b]["y"] for b in range(N_CORES)], axis=0
    ).astype(np.float32)
